# revision 1
# baseline (speedup 1.0000x reference)
"""Multi-head attention (B=4, S=2048, D=1024, H=16) on 8 TRN2 NeuronCores.

Sharding: data-parallel over batch (4) x tensor-parallel over head halves (2).
Core c handles batch b = c//2 and heads [8g, 8g+8) where g = c%2. Each core
computes a partial [S, D] output-projection contribution; the host sums the
two head-group partials per batch.

Schedule: the scalar engine's exp() stream (256 x [128,1024] activations,
~270us busy) is the hard floor, so everything is built to keep it saturated:

  - 256 global "steps", one exp each: (qc, hp-pair-of-pairs, kt, parity).
    Per step the PE does one score pair-slot (row-tiled heads, 215ns), one
    P@V pair-slot (col-tiled heads at M=64) and half of a 4-wide d-slot.
  - softmax denominators come from quad-col-tiled ones-matmuls (four M=1
    matmuls at col strips 0/32/64/96 sharing one 512-cycle stream), each
    head's d accumulating into a single PSUM row over all 16 k-tiles.
    U accumulates fully in PSUM (no DVE folding). PSUM has_written clears
    are per-element-on-write, so every first write carries start=True.
  - projections (Q/K/V/w_o) are chopped into ~4-matmul units injected
    between attention slots with staggered deadlines + steady pacing, so
    no single step overruns the PE slack and stalls the exp stream.
  - prologue-critical tensors (wk,xk0 / wq,xq0) ride two independent HWDGE
    queues (sync + scalar) as per-d-slice DMAs so the first K/Q projection
    matmuls start as soon as their slices land.

PSUM budget (8 banks): 2 x sc [128,1024] (4) + 2 x U [128,512] (2) +
d [128,512] (1) + proj accumulator (1).
"""

import heapq

import numpy as np

B, S, D, H = 4, 2048, 1024, 16
DK = D // H          # 64
G = 2                # head groups (tensor-parallel degree per batch)
HL = H // G          # 8 local heads per core
DV = HL * DK         # 512 local value dim
N_CORES = 8
NQC = 4              # q-chunks of 512
NKT = 16             # k-tiles of 128

_cached = {}


def _build():
    import concourse.bass as bass
    import concourse.tile as tile
    from concourse import bacc, mybir

    f32 = mybir.dt.float32
    bf16 = mybir.dt.bfloat16
    EXP = mybir.ActivationFunctionType.Exp

    nc = bacc.Bacc("TRN2", target_bir_lowering=False, debug=False,
                   num_devices=N_CORES)

    # All host-packed so big DMAs get 8KB-contiguous rows.
    wkW = nc.dram_tensor("wkW", [128, 4096], bf16, kind="ExternalInput").ap()
    wqW = nc.dram_tensor("wqW", [128, 4096], bf16, kind="ExternalInput").ap()
    wvW = nc.dram_tensor("wvW", [128, 4096], bf16, kind="ExternalInput").ap()
    woW = nc.dram_tensor("woW", [128, 4096], bf16, kind="ExternalInput").ap()
    xkC = [nc.dram_tensor(f"xk{c}", [128, 4096], bf16,
                          kind="ExternalInput").ap() for c in range(4)]
    xvC = [nc.dram_tensor(f"xv{c}", [128, 4096], bf16,
                          kind="ExternalInput").ap() for c in range(4)]
    xqC = [nc.dram_tensor(f"xq{c}", [128, 4096], bf16,
                          kind="ExternalInput").ap() for c in range(4)]
    out = nc.dram_tensor("out", [S, D], f32, kind="ExternalOutput").ap()

    with tile.TileContext(nc) as tc:
        with (
            tc.tile_pool(name="persist", bufs=1) as persist,
            tc.tile_pool(name="xpool", bufs=1) as xpool,
            tc.tile_pool(name="ppool", bufs=4) as ppool,
            tc.tile_pool(name="work", bufs=2) as work,
            tc.tile_pool(name="psum", bufs=1,
                         space=bass.MemorySpace.PSUM) as psum,
        ):
            KT = {}      # (t, c) -> [128, 512] bf16 K^T tiles
            QT = {}      # (t, qc) -> [128, 512]
            VT = {}      # kt -> [128, 512] (all 8 heads' V columns)
            OT = {}      # (qc, hp) -> [128, 512] normalized out^T
            Ubank = {}   # parity -> live U psum tile
            Dbank = {}   # live d psum tile
            wsb = {}
            xk_sb, xv_sb, xq_sb = {}, {}, {}

            # ---------------- DMA staging ----------------
            def load_w(name, src, engine=None, sliced=False):
                t = persist.tile([128, 4096], bf16, tag=f"w_{name}", name="w")
                e = engine or nc.sync
                if sliced:
                    for blk in range(4):
                        e.dma_start(t[:, 1024 * blk:1024 * (blk + 1)],
                                    src[:, 1024 * blk:1024 * (blk + 1)])
                else:
                    e.dma_start(t[:], src[:, :])
                wsb[name] = t

            def load_x(dst, key, src, c, engine, bufs, sliced=False):
                t = xpool.tile([128, 4096], bf16, tag=f"x_{key}", name="x",
                               bufs=bufs)
                if sliced:
                    for d in range(8):
                        engine.dma_start(t[:, 512 * d:512 * (d + 1)],
                                        src[:, 512 * d:512 * (d + 1)])
                else:
                    engine.dma_start(t[:], src[:, :])
                dst[c] = t

            # ---------------- projection pieces (split into units) ------
            def proj_units(wname, xt, t, done):
                # two 4-matmul units accumulating [128,512]; done(acc) last
                cell = {}

                def u1():
                    cell["acc"] = psum.tile([128, 512], f32, tag="pacc",
                                            name="pacc")
                    for d in range(4):
                        nc.tensor.matmul(
                            cell["acc"][:],
                            wsb[wname][:, 1024 * t + 128 * d:
                                       1024 * t + 128 * (d + 1)],
                            xt[:, 512 * d:512 * (d + 1)],
                            start=(d == 0), stop=False)

                def u2():
                    for d in range(4, 8):
                        nc.tensor.matmul(
                            cell["acc"][:],
                            wsb[wname][:, 1024 * t + 128 * d:
                                       1024 * t + 128 * (d + 1)],
                            xt[:, 512 * d:512 * (d + 1)],
                            start=False, stop=(d == 7))
                    done(cell["acc"])
                return [u1, u2]

            def k_units(t, c):
                def done(acc):
                    kt_ = persist.tile([128, 512], bf16, tag=f"kT{t}_{c}",
                                       name="kT")
                    nc.vector.tensor_copy(kt_[:], acc[:])
                    KT[(t, c)] = kt_
                return proj_units("wk", xk_sb[c], t, done)

            def q_units(t, qc):
                def done(acc):
                    qt_ = persist.tile([128, 512], bf16, tag=f"qT{t}",
                                       name="qT", bufs=2)
                    nc.vector.tensor_copy(qt_[:], acc[:])
                    QT[(t, qc)] = qt_
                return proj_units("wq", xq_sb[qc], t, done)

            def v_units(kt, w):
                c, j = kt // 4, kt % 4
                cell = {}

                def u():
                    cell["acc"] = psum.tile([128, 512], f32,
                                            tag="pacc", name="pacc")
                    for d in range(8):
                        nc.tensor.matmul(
                            cell["acc"][:, 0:256],
                            xv_sb[c][:, 512 * d + 128 * j:
                                      512 * d + 128 * (j + 1)],
                            wsb["wv"][:, 512 * d + 256 * w:
                                      512 * d + 256 * (w + 1)],
                            start=(d == 0), stop=(d == 7))
                    if kt not in VT:
                        VT[kt] = persist.tile(
                            [128, 512], bf16, tag=f"v{kt}", name="v")
                    nc.vector.tensor_copy(
                        VT[kt][:, 256 * w:256 * (w + 1)],
                        cell["acc"][:, 0:256])
                return [u]

            def wo_units(qc, st, ncol, acc_tag="pacc", trange=(0, 4),
                         dest=None):
                cell = {}
                tlo0, thi0 = trange
                dst = out if dest is None else dest
                row0 = (512 * qc if dest is None else 0) + 128 * st

                def mk(tlo, thi, first, last):
                    def u():
                        if first:
                            cell["acc"] = psum.tile(
                                [128, 512], f32, tag=acc_tag, name="pacc",
                                bufs=(2 if acc_tag == "sc" else None))
                        for t in range(tlo, thi):
                            nc.tensor.matmul(
                                cell["acc"][:],
                                OT[(qc, t)][:, 128 * st:128 * (st + 1)],
                                wsb["wo"][:, 1024 * t + 512 * ncol:
                                          1024 * t + 512 * (ncol + 1)],
                                start=(t == tlo and first),
                                stop=(t == thi - 1 and last))
                        if last:
                            ob = work.tile([128, 512], f32, tag="ob",
                                           name="ob", bufs=2)
                            nc.vector.tensor_copy(ob[:], cell["acc"][:])
                            nc.sync.dma_start(
                                dst[row0:row0 + 128,
                                    512 * ncol:512 * (ncol + 1)], ob[:])
                    return u
                if thi0 - tlo0 <= 2:
                    return [mk(tlo0, thi0, True, True)]
                mid = (tlo0 + thi0) // 2
                return [mk(tlo0, mid, True, False), mk(mid, thi0, False, True)]

            UNIT_COST = {"k": 880.0, "q": 880.0, "v": 940.0, "wo": 450.0}

            # ---------------- attention machinery ----------------
            ones = persist.tile([128, 1], bf16, tag="ones", name="ones")
            nc.vector.tensor_copy(
                ones[:], nc.const_aps.tensor(1.0, (128, 1), bf16))

            P_of = {}    # step -> P tile
            sc_q = {}

            def step_ids(u):
                return (u // 64, (u // 32) % 2, 2 * ((u // 32) % 2) + u % 2,
                        (u % 32) // 2, u % 2)

            def emit_score(u):
                qc, hpp, hp, kt, par = step_ids(u)
                c, j = kt // 4, kt % 4
                sc = psum.tile([128, 1024], f32, tag="sc", name="sc", bufs=2)
                for i in range(2):
                    po = 64 * i
                    nc.tensor.matmul(
                        sc[:, 512 * i:512 * (i + 1)],
                        KT[(hp, c)][po:po + 64, 128 * j:128 * (j + 1)],
                        QT[(hp, qc)][po:po + 64, :],
                        start=True, stop=True)
                sc_q[u] = sc

            def emit_exp(u):
                p = ppool.tile([128, 1024], bf16, tag="p", name="p")
                nc.scalar.activation(p[:], sc_q.pop(u)[:], EXP, scale=0.125)
                P_of[u] = p

            def emit_pv(u):
                qc, hpp, hp, kt, par = step_ids(u)
                if kt == 0:
                    Ubank[par] = psum.tile([128, 512], f32, tag="u",
                                           name="u", bufs=2)
                U = Ubank[par]
                p = P_of[u]
                for i in range(2):
                    nc.tensor.matmul(
                        U[64 * i:64 * (i + 1), :],
                        VT[kt][:, 128 * hp + 64 * i:128 * hp + 64 * (i + 1)],
                        p[:, 512 * i:512 * (i + 1)],
                        start=(kt == 0), stop=(kt == 15))

            def emit_dquad(u):
                # u odd: P(u-1) = even parity, P(u) = odd parity, same kt
                qc, hpp, hp, kt, par = step_ids(u)
                if kt == 0:
                    Dbank["d"] = psum.tile([128, 512], f32, tag="d", name="d")
                db = Dbank["d"]
                srcs = [P_of[u - 1][:, 0:512], P_of[u - 1][:, 512:1024],
                        P_of[u][:, 0:512], P_of[u][:, 512:1024]]
                for idx, src in enumerate(srcs):
                    nc.tensor.matmul(
                        db[32 * idx:32 * idx + 1, :], ones[:], src,
                        start=(kt == 0), stop=(kt == 15),
                        tile_position=(0, 32 * idx))
                P_of.pop(u - 1, None)
                P_of.pop(u, None)

            Usb_of = {}

            def emit_ucopy(par):
                usb = work.tile([128, 512], f32, tag=f"usb{par}", name="usb",
                                bufs=2)
                nc.vector.tensor_copy(usb[:], Ubank[par][:])
                Usb_of[par] = usb

            def emit_normalize(qc, hpp):
                dsb = work.tile([128, 512], f32, tag="dsb", name="dsb",
                                bufs=2)
                nc.vector.tensor_copy(dsb[:], Dbank["d"][:])
                for par in range(2):
                    hp = 2 * hpp + par
                    ot = persist.tile([128, 512], bf16, tag=f"oT{hp}",
                                      name="oT", bufs=2)
                    OT[(qc, hp)] = ot
                    usb = Usb_of[par]
                    for i in range(2):
                        row = 64 * par + 32 * i
                        dr = work.tile([1, 512], f32, tag="dr", name="dr",
                                       bufs=2)
                        nc.sync.dma_start(dr[:], dsb[row:row + 1, :])
                        rr = work.tile([1, 512], f32, tag="rr", name="rr",
                                       bufs=2)
                        nc.vector.reciprocal_approx_fast(rr[:], dr[:])
                        if i == 0:
                            rb = work.tile([64, 512], f32, tag="rb0",
                                           name="rb", bufs=1)
                            nc.gpsimd.partition_broadcast(rb[:], rr[:])
                            nc.vector.tensor_mul(ot[0:64, :], usb[0:64, :],
                                                 rb[:])
                        else:
                            rb = work.tile([128, 512], f32, tag="rb1",
                                           name="rb", bufs=1)
                            nc.gpsimd.partition_broadcast(rb[:], rr[:])
                            nc.vector.tensor_mul(ot[64:128, :],
                                                 usb[64:128, :],
                                                 rb[64:128, :])

            # ---------------- emission schedule ----------------
            # scalar-engine ACT table preload (runs during DMA prologue)
            jin = work.tile([128, 8], f32, tag="jin", name="jin", bufs=1)
            nc.vector.tensor_copy(jin[:],
                                  nc.const_aps.tensor(0.0, (128, 8), f32))
            jout = work.tile([128, 8], bf16, tag="jout", name="jout", bufs=1)
            nc.scalar.activation(jout[:], jin[:], EXP)

            # prologue-critical DMAs: per-d slices on two queues so the
            # first K/Q projection matmuls start as slices land
            load_w("wk", wkW, nc.sync, sliced=True)
            load_x(xk_sb, "k", xkC[0], 0, nc.sync, 4, sliced=True)
            load_w("wq", wqW, nc.scalar, sliced=True)
            load_x(xq_sb, "q", xqC[0], 0, nc.scalar, 2, sliced=True)
            load_w("wv", wvW)
            load_x(xv_sb, "v", xvC[0], 0, nc.sync, 4)
            load_x(xq_sb, "q", xqC[1], 1, nc.scalar, 2)
            for c in range(1, 4):
                load_x(xk_sb, "k", xkC[c], c, nc.sync, 4)
                load_x(xv_sb, "v", xvC[c], c, nc.sync, 4)
            load_w("wo", woW)
            load_x(xq_sb, "q", xqC[2], 2, nc.sync, 2)
            load_x(xq_sb, "q", xqC[3], 3, nc.sync, 2)

            # unit heap: (deadline_step, seq, cost, fn)
            pieces = []
            seq = [0]
            TOTAL = [0.0]

            def push_units(dl, kind, units):
                for i, u in enumerate(units):
                    heapq.heappush(pieces,
                                   (dl + i, seq[0], UNIT_COST[kind], u))
                    seq[0] += 1
                    TOTAL[0] += UNIT_COST[kind]

            for t in range(4):
                for c in range(4):
                    if (t, c) in ((0, 0), (1, 0)):
                        continue
                    u0 = 32 * (t // 2) + 8 * c + (t % 2)
                    push_units(u0 - 5, "k", k_units(t, c))
            for qc in range(NQC):
                for t in range(4):
                    if (t, qc) in ((0, 0), (1, 0)):
                        continue
                    u0 = 64 * qc + 32 * (t // 2) + (t % 2)
                    push_units(u0 - 5, "q", q_units(t, qc))
            for w in range(2):
                for kt in range(NKT):
                    if w == 0 and kt in (0, 1):
                        continue
                    push_units(32 * w + 2 * kt - 3, "v", v_units(kt, w))

            injected = [0.0]
            CAP = 600.0      # ns of injected PE work per step

            def inject(s):
                while pieces and (pieces[0][0] <= s
                                  or injected[0] < (s + 1) * CAP):
                    _, _, cost, fn = heapq.heappop(pieces)
                    fn()
                    injected[0] += cost

            # prologue compute: steps 0 and 1
            for un in k_units(0, 0) + q_units(0, 0):
                un()
            emit_score(0)
            for un in k_units(1, 0) + q_units(1, 0):
                un()
            emit_score(1)
            for un in v_units(0, 0) + v_units(1, 0):
                un()

            NSTEP = 256
            for g in range(NSTEP // 2 + 1):
                s0, s1 = 2 * g - 2, 2 * g - 1   # PV steps this group
                if s0 >= 0:
                    emit_pv(s0)
                    if s0 % 32 == 30:
                        emit_ucopy(0)
                    emit_pv(s1)
                    emit_dquad(s1)
                    if s1 % 32 == 31:
                        emit_ucopy(1)
                        qc, hpp, _, _, _ = step_ids(s1)
                        emit_normalize(qc, hpp)
                        if qc < 3 and hpp == 1:
                            base = 64 * (qc + 1)
                            for st in range(4):
                                for ncol in range(2):
                                    push_units(base + 4 + 3 * (2 * st + ncol),
                                               "wo", wo_units(qc, st, ncol))
                if 2 * g + 2 < NSTEP:
                    emit_score(2 * g + 2)
                    emit_score(2 * g + 3)
                if 2 * g < NSTEP:
                    emit_exp(2 * g)
                    emit_exp(2 * g + 1)
                inject(2 * g + 1)

            # drain: remaining units, then qc3's w_o with alternating
            # accumulator tags (sc banks are free now) to avoid cast
            # serialization in the tail
            while pieces:
                _, _, _, fn = heapq.heappop(pieces)
                fn()
            for st in range(4):
                for ncol in range(2):
                    tag = "pacc" if (2 * st + ncol) % 2 == 0 else "sc"
                    for un in wo_units(3, st, ncol, acc_tag=tag):
                        un()

    nc.compile()
    return nc


def make_in_maps(query, key, value, w_q, w_k, w_v, w_o):
    import ml_dtypes
    bf = ml_dtypes.bfloat16

    def c(a):
        return np.ascontiguousarray(a).astype(bf)

    def pack_w(wT, blocks, width):
        # [blocks*128, width] -> [128, blocks*width] (d-tiles side by side)
        return c(wT.reshape(blocks, 128, width).transpose(1, 0, 2)
                 .reshape(128, blocks * width))

    def pack_tmaj(wT):
        # [1024(d), 512(dk)] -> [128, 4096], col = 1024*t + 128*d + dk_local
        return c(wT.reshape(8, 128, 4, 128).transpose(1, 2, 0, 3)
                 .reshape(128, 4096))

    def pack_x(xT):
        # xT [D, S] -> per k/q-chunk [128, 4096] (8 d-tiles side by side)
        outs = []
        for ch in range(4):
            sl = xT[:, 512 * ch:512 * (ch + 1)]           # [1024, 512]
            outs.append(pack_w(sl, 8, 512))
        return outs

    in_maps = []
    for core in range(N_CORES):
        b, g = core // G, core % G
        rows = slice(DV * g, DV * (g + 1))
        xq = pack_x(np.asarray(query[b], np.float32).T)
        xk = pack_x(np.asarray(key[b], np.float32).T)
        xv = pack_x(np.asarray(value[b], np.float32).T)
        m = {
            "wqW": pack_tmaj(np.asarray(w_q[rows, :], np.float32).T),
            "wkW": pack_tmaj(np.asarray(w_k[rows, :], np.float32).T),
            "wvW": pack_w(np.asarray(w_v[rows, :], np.float32).T, 8, 512),
            "woW": pack_w(np.asarray(w_o[:, rows], np.float32).T, 4, 1024),
        }
        for ch in range(4):
            m[f"xq{ch}"] = xq[ch]
            m[f"xk{ch}"] = xk[ch]
            m[f"xv{ch}"] = xv[ch]
        in_maps.append(m)
    return in_maps


def kernel(query, key, value, w_q, w_k, w_v, w_o):
    from concourse.bass_utils import run_bass_kernel_spmd

    if "nc" not in _cached:
        _cached["nc"] = _build()
    nc = _cached["nc"]

    in_maps = make_in_maps(query, key, value, w_q, w_k, w_v, w_o)
    res = run_bass_kernel_spmd(nc, in_maps, list(range(N_CORES)))
    full = np.empty((B, S, D), np.float32)
    for b in range(B):
        full[b] = res.results[G * b]["out"] + res.results[G * b + 1]["out"]
    return full

